# revision 26
# baseline (speedup 1.0000x reference)
"""GCNConv(flow=target_to_source) + BatchNorm + ReLU + residual, on 8 trn2 NeuronCores.

Math: with self-loops appended to the edge list,
    deg[i]   = #{e : row[e] == i}
    dinv     = deg ** -0.5
    norm_e   = dinv[row_e] * dinv[col_e]
    S[r]     = sum_{e: row[e]=r} norm_e * x[col_e]     (dma_gather + onehot-matmul)
    out      = S @ W                                   (W commutes past the aggregation)
    y        = relu((out - mean) * rsqrt(var + eps) * gamma + beta) + x
(b cancels inside BatchNorm, so it is dropped.)

v2 vs v1: no bf16 v-table is built on device.  The gather reads x_bf directly
and the full GCN norm (dinv[row]*dinv[col]) is folded into host-precomputed
"onehot" scatter matrices (bf16 values at (edge_slot, target_row), zero on pad
slots) — standard GCN edge-weight preprocessing.  This removes the serial
v-build phase and the DVE-hostile stride-0 is_equal onehot build (DVE runs at
1 elem/cycle when any operand has a stride-0 last dim).  Tile counts are exact
per (block, lo/hi) segment maxed across cores (SPMD shares one program).
Appended self-loops are not gathered: each block gets one "self" tile filled
by a plain strided DMA of the core's own rows plus a diagonal dinv^2 onehot
slab (-10% gather descriptors).  Each gather call is split across two SWDGE
queues (_QSPLIT=2) to overlap Q7 descriptor generation, the dominant gather
cost (~3ns/descriptor).

Sharding: nodes (rows) split across 8 cores; edges partitioned by destination
row so the scatter-add is core-local PSUM accumulation.  BN statistics go
through a [128,2] AllReduce.

dma_gather takes int16 indices, so x_bf is addressed as two halves
(lo: rows < SPLIT, hi: rows >= SPLIT); each block's edges are ordered
lo-cols-first.  Index buffers are packed in the HW layout: idx i at
(partition i%16, column i//16), replicated across the eight 16-partition
groups.
"""

import os
import sys

sys.path.insert(0, "/opt/trn_rl_repo")
os.environ.setdefault("MYCRO_LOCAL_CACHE", "1")

from contextlib import ExitStack

import ml_dtypes
import numpy as np

CORES = 8
BN_EPS = 1e-5
SPLIT = 32768
_REP = 1  # debug/timing: repeat the whole compute body inside one program
_FAKE_GATHER = False  # timing-only: plain DMA instead of dma_gather (wrong data)
_SKIP_CC = False      # timing-only: skip the AllReduce (wrong stats)
_QSPLIT = 2  # split each gather call into this many queue-parallel pieces
_NSLAB = 1   # final apply/store pipelining granularity (1 = monolithic)
_GBUFS = 2   # gather-pool depth (3 showed no robust win over 2)
_PSBUFS = 2  # ps_main depth (PSUM is 8 banks: 2x2 main + 2 stat + 2 misc)
_CACHE: dict = {}


def _pick_blk(npc: int) -> int:
    for blk in range(125, 0, -1):
        if npc % blk == 0:
            return blk
    raise ValueError(npc)


def _strided(ap_src, offset_elems, dims):
    import concourse.bass as bass

    return bass.AP(ap_src.tensor, offset_elems, [list(d) for d in dims])


def _build_nc(N, D, NPC, BLK, NBLK, SUP, SPL, XF_PAD, XLB_PAD,
              t_lo, t_hi, LO_COLS, HI_COLS, TOT_TILES):
    """t_lo/t_hi: per-local-block tile counts (tuples of len NBLK).

    Appended self-loops are NOT gathered: each block has one extra "self"
    tile filled by a plain strided DMA from x_loc_bf (the core's own rows),
    with a diagonal onehot slab carrying dinv[i]^2.  Slab order per block:
    lo tiles, hi tiles, self tile."""
    from concourse import bacc, bass, mybir, tile
    from concourse.masks import make_identity

    f32 = mybir.dt.float32
    bf16 = mybir.dt.bfloat16
    i16 = mybir.dt.int16

    nc = bacc.Bacc(
        "TRN2",
        target_bir_lowering=False,
        debug=False,
        enable_asserts=False,
        num_devices=CORES,
        num_swdge_queues=4,
    )

    x_bf = nc.dram_tensor("x_bf", [XF_PAD, D], bf16, kind="ExternalInput").ap()
    xlb_t = nc.dram_tensor("x_loc_bf", [XLB_PAD, D], bf16, kind="ExternalInput").ap()
    lo_t = nc.dram_tensor("lo_idx", [128, LO_COLS], i16, kind="ExternalInput").ap()
    hi_t = nc.dram_tensor("hi_idx", [128, HI_COLS], i16, kind="ExternalInput").ap()
    oh_t = nc.dram_tensor("oh_arr", [128, TOT_TILES * BLK], bf16, kind="ExternalInput").ap()
    xloc_t = nc.dram_tensor("x_loc", [NPC, D], f32, kind="ExternalInput").ap()
    w_t = nc.dram_tensor("w_mat", [D, D], bf16, kind="ExternalInput").ap()
    gamma_t = nc.dram_tensor("gamma", [D], f32, kind="ExternalInput").ap()
    beta_t = nc.dram_tensor("beta", [D], f32, kind="ExternalInput").ap()
    y_t = nc.dram_tensor("y_out", [NPC, D], f32, kind="ExternalOutput").ap()

    # per-chunk prefix offsets (slots are 128-aligned so /16 cols are exact)
    chunks = []
    lo_slot0 = hi_slot0 = oh_tile0 = 0
    for c0 in range(0, NBLK, SUP):
        blocks = list(range(c0, min(c0 + SUP, NBLK)))
        LT = sum(t_lo[b] for b in blocks)
        HT = sum(t_hi[b] for b in blocks)
        chunks.append((blocks, LT, HT, lo_slot0, hi_slot0, oh_tile0))
        lo_slot0 += LT * 128
        hi_slot0 += HT * 128
        oh_tile0 += LT + HT + len(blocks)  # + self slabs

    with tile.TileContext(nc) as tc, ExitStack() as ctx:
        const = ctx.enter_context(tc.tile_pool(name="const", bufs=1))
        gath = ctx.enter_context(tc.tile_pool(name="gath", bufs=_GBUFS))
        ohp = ctx.enter_context(tc.tile_pool(name="ohp", bufs=2))
        evp = ctx.enter_context(tc.tile_pool(name="evp", bufs=3))
        big = ctx.enter_context(tc.tile_pool(name="big", bufs=1))
        ps_main = ctx.enter_context(tc.tile_pool(name="ps_main", bufs=_PSBUFS, space="PSUM"))
        ps_stat = ctx.enter_context(tc.tile_pool(name="ps_stat", bufs=1, space="PSUM"))
        ps_misc = ctx.enter_context(tc.tile_pool(name="ps_misc", bufs=2, space="PSUM"))
        dram = ctx.enter_context(tc.tile_pool(name="dram", bufs=1, space="DRAM"))

        # ---- constants -----------------------------------------------------
        w_sb = const.tile([D, D], bf16)
        nc.sync.dma_start(w_sb[:], w_t[:])
        lo_sb = const.tile([128, LO_COLS], i16)
        nc.sync.dma_start(lo_sb[:], lo_t[:])
        hi_sb = const.tile([128, HI_COLS], i16)
        nc.sync.dma_start(hi_sb[:], hi_t[:])
        ones_sb = const.tile([128, 1], f32)
        nc.vector.memset(ones_sb[:], 1.0)
        onesrow_sb = const.tile([1, 128], f32)
        nc.vector.memset(onesrow_sb[:], 1.0)
        gb_sb = const.tile([128, 2], f32)
        nc.sync.dma_start(gb_sb[:, 0:1], gamma_t[:, None])
        nc.sync.dma_start(gb_sb[:, 1:2], beta_t[:, None])
        ident_sb = const.tile([128, 128], f32)
        make_identity(nc, ident_sb[:])

        # residual x, loaded early (independent of everything else)
        xl = big.tile([128, NBLK * D], f32)
        nc.sync.dma_start(
            xl[:BLK, :], _strided(xloc_t, 0, [[D, BLK], [BLK * D, NBLK], [1, D]])
        )

        for _rep in range(_REP):
            _sfx = f"_r{_rep}"
            out_all = big.tile([128, NBLK * D], f32, name="out_all" + _sfx)
            s1 = ps_stat.tile([128, 1], f32, tag="s1")
            s2 = ps_stat.tile([128, 1], f32, tag="s2")
            for ci, (blocks, LT, HT, lo_s0, hi_s0, oh_t0) in enumerate(chunks):
                nb = len(blocks)
                TT = LT + HT
                qq = ci * 2
                g = gath.tile([128, TT + nb, D], bf16, tag="g")
                # self tiles: block rows via plain strided DMA (all 128
                # partitions so no stale SBUF reaches the matmul; onehot is
                # zero beyond row BLK)
                nc.sync.dma_start(
                    g[:, TT:TT + nb, :],
                    _strided(xlb_t, blocks[0] * BLK * D,
                             [[D, 128], [BLK * D, nb], [1, D]]),
                )
                if _FAKE_GATHER:
                    nc.sync.dma_start(
                        g[:, 0:TT, :],
                        _strided(x_bf, 0, [[D, 128], [128 * D, TT], [1, D]]),
                    )
                else:
                    def issue(tile0, ntiles, src_ap, idx_sb, slot0, qbase, qstep):
                        pieces = _QSPLIT if ntiles >= _QSPLIT else max(ntiles, 1)
                        tb = 0
                        for pi in range(pieces):
                            nt = ntiles // pieces + (1 if pi < ntiles % pieces else 0)
                            if nt == 0:
                                continue
                            s0 = slot0 + tb * 128
                            nc.gpsimd.dma_gather(
                                g[:, tile0 + tb:tile0 + tb + nt, :],
                                src_ap,
                                idx_sb[:, s0 // 16:(s0 + nt * 128) // 16],
                                nt * 128,
                                nt * 128,
                                D,
                                single_packet=False,
                                queue_num=(qbase + pi * qstep) % 4,
                            )
                            tb += nt

                    if LT:
                        issue(0, LT, x_bf[0:SPL, :], lo_sb, lo_s0, qq, 2)
                    if HT:
                        issue(LT, HT, x_bf[SPL:XF_PAD, :], hi_sb, hi_s0, qq + 1, 2)
                oh = ohp.tile([128, (TT + nb) * BLK], bf16, tag="oh")
                nc.sync.dma_start(
                    oh[:], oh_t[:, oh_t0 * BLK:(oh_t0 + TT + nb) * BLK]
                )
                # onehot slabs are ordered block-major: for each block its lo
                # tiles then its hi tiles.
                slab = 0
                lo_base = 0
                hi_base = LT
                for bj, b in enumerate(blocks):
                    tl, th = t_lo[b], t_hi[b]
                    gidx = [lo_base + t for t in range(tl)] + \
                           [hi_base + t for t in range(th)] + [TT + bj]
                    lo_base += tl
                    hi_base += th
                    st = ps_main.tile([128, BLK], f32, tag="st")
                    for k, gi in enumerate(gidx):
                        nc.tensor.matmul(
                            out=st[:], lhsT=g[:, gi, :],
                            rhs=oh[:, slab * BLK:(slab + 1) * BLK],
                            start=(k == 0), stop=(k == len(gidx) - 1),
                        )
                        slab += 1
                    stb = evp.tile([128, BLK], bf16, tag="stb")
                    nc.vector.tensor_copy(out=stb[:], in_=st[:])
                    ow = ps_main.tile([BLK, D], f32, tag="ow")
                    nc.tensor.matmul(out=ow[:], lhsT=stb[:], rhs=w_sb[:],
                                     start=True, stop=True)
                    oslice = out_all[:BLK, b * D:(b + 1) * D]
                    nc.scalar.copy(out=oslice, in_=ow[:])
                    sq_s = evp.tile([128, D], f32, tag="sq")
                    nc.scalar.square(out=sq_s[:BLK, :], in_=oslice)
                    nc.tensor.matmul(
                        out=s1[:], lhsT=oslice, rhs=ones_sb[:BLK, :],
                        start=(b == 0), stop=(b == NBLK - 1),
                    )
                    nc.tensor.matmul(
                        out=s2[:], lhsT=sq_s[:BLK, :], rhs=ones_sb[:BLK, :],
                        start=(b == 0), stop=(b == NBLK - 1),
                    )

            # ---- BN stats AllReduce + affine params ----------------------------
            stat_sb = const.tile([128, 2], f32, name="stat_sb" + _sfx)
            nc.vector.tensor_copy(out=stat_sb[:, 0:1], in_=s1[:])
            nc.vector.tensor_copy(out=stat_sb[:, 1:2], in_=s2[:])
            cc_in = dram.tile([128, 2], f32)
            cc_out = dram.tile([128, 2], f32, addr_space="Shared")
            statg = const.tile([128, 2], f32, name="statg" + _sfx)
            if _SKIP_CC:
                nc.vector.tensor_copy(out=statg[:], in_=stat_sb[:])
            else:
                nc.sync.dma_start(cc_in[:], stat_sb[:])
                nc.gpsimd.collective_compute(
                    "AllReduce",
                    mybir.AluOpType.add,
                    replica_groups=[list(range(CORES))],
                    ins=[cc_in.opt()],
                    outs=[cc_out.opt()],
                )
                nc.sync.dma_start(statg[:], cc_out[:])

            invn = 1.0 / float(N)
            mean = const.tile([128, 1], f32, name="mean" + _sfx)
            nc.vector.tensor_scalar(
                out=mean[:], in0=statg[:, 0:1], scalar1=invn, scalar2=None,
                op0=mybir.AluOpType.mult,
            )
            vareps = const.tile([128, 1], f32, name="vareps" + _sfx)
            m2 = const.tile([128, 1], f32, name="m2" + _sfx)
            nc.vector.tensor_tensor(out=m2[:], in0=mean[:], in1=mean[:], op=mybir.AluOpType.mult)
            nc.vector.tensor_scalar(
                out=vareps[:], in0=statg[:, 1:2], scalar1=invn, scalar2=BN_EPS,
                op0=mybir.AluOpType.mult, op1=mybir.AluOpType.add,
            )
            nc.vector.tensor_tensor(
                out=vareps[:], in0=vareps[:], in1=m2[:], op=mybir.AluOpType.subtract
            )
            rec1 = const.tile([128, 1], f32, name="rec1" + _sfx)
            nc.vector.reciprocal(out=rec1[:], in_=vareps[:])
            rsq = const.tile([128, 1], f32, name="rsq" + _sfx)
            nc.scalar.sqrt(out=rsq[:], in_=rec1[:])
            ab_sb = const.tile([128, 2], f32, name="ab_sb" + _sfx)
            nc.vector.tensor_tensor(
                out=ab_sb[:, 0:1], in0=rsq[:], in1=gb_sb[:, 0:1], op=mybir.AluOpType.mult
            )
            tmb = const.tile([128, 1], f32, name="tmb" + _sfx)
            nc.vector.tensor_tensor(
                out=tmb[:], in0=mean[:], in1=ab_sb[:, 0:1], op=mybir.AluOpType.mult
            )
            nc.vector.tensor_tensor(
                out=ab_sb[:, 1:2], in0=gb_sb[:, 1:2], in1=tmb[:], op=mybir.AluOpType.subtract
            )

            def bcast_col(col_ap, nm):
                tp = ps_misc.tile([128, 128], f32, tag="m")
                nc.tensor.transpose(out=tp[:1, :], in_=col_ap, identity=ident_sb[:])
                rowt = const.tile([1, 128], f32, name=f"rowt_{nm}" + _sfx)
                nc.vector.tensor_copy(out=rowt[:], in_=tp[:1, :])
                bc_ps = ps_misc.tile([128, 128], f32, tag="m")
                nc.tensor.matmul(out=bc_ps[:], lhsT=onesrow_sb[:], rhs=rowt[:], start=True, stop=True)
                bc = const.tile([128, 128], f32, name=f"bc_{nm}" + _sfx)
                nc.vector.tensor_copy(out=bc[:], in_=bc_ps[:])
                return bc

            a_bc = bcast_col(ab_sb[:, 0:1], "a")
            b_bc = bcast_col(ab_sb[:, 1:2], "b")

            # ---- final apply: y = relu(out*A + B) + x --------------------------
            # per-slab so each slab's y store overlaps the next slab's apply
            SB = NBLK // _NSLAB if NBLK % _NSLAB == 0 else NBLK
            for s0 in range(0, NBLK, SB):
                cols = slice(s0 * D, (s0 + SB) * D)
                a_rep = _strided(a_bc[:], 0, [[a_bc[:].ap[0][0], BLK], [0, SB], [1, D]])
                b_rep = _strided(b_bc[:], 0, [[b_bc[:].ap[0][0], BLK], [0, SB], [1, D]])
                oc = out_all[:BLK, cols]
                nc.vector.tensor_tensor(out=oc, in0=oc, in1=a_rep, op=mybir.AluOpType.mult)
                nc.vector.tensor_tensor(out=oc, in0=oc, in1=b_rep, op=mybir.AluOpType.add)
                nc.vector.tensor_scalar(
                    out=oc, in0=oc, scalar1=0.0, scalar2=None, op0=mybir.AluOpType.max,
                )
                nc.vector.tensor_tensor(
                    out=oc, in0=oc, in1=xl[:BLK, cols], op=mybir.AluOpType.add,
                )
                nc.sync.dma_start(
                    _strided(y_t, s0 * BLK * D, [[D, BLK], [BLK * D, SB], [1, D]]), oc
                )

    nc.compile()
    return nc


def _pack_idx(stream):
    """Pack a flat slot stream (len multiple of 128) into the dma_gather int16
    layout: idx i -> (partition i%16, col i//16), replicated across the 8
    groups of 16 partitions.  Returns [128, len//16] int16."""
    assert len(stream) % 128 == 0
    grid = stream.reshape(-1, 16).T  # [16, ncols]
    return np.tile(grid, (8, 1))


def prepare(x, edge_index, W, b, gamma, beta):
    x = np.asarray(x, np.float32)
    W = np.asarray(W, np.float32)
    gamma = np.asarray(gamma, np.float32)
    beta = np.asarray(beta, np.float32)
    N, D = x.shape
    assert N % CORES == 0
    NPC = N // CORES
    BLK = _pick_blk(NPC)
    NBLK = NPC // BLK
    SUP = 1
    for s in (5, 4, 3, 2):
        if NBLK % s == 0:
            SUP = s
            break
    SPL = min(SPLIT, N)
    XF_PAD = ((N + 127) // 128) * 128

    row = np.asarray(edge_index[0]).astype(np.int64)
    col = np.asarray(edge_index[1]).astype(np.int64)
    # degree counts the appended self-loops, but the loops themselves are
    # handled by a per-block DMA + diagonal onehot slab (not gathered)
    rows_all = np.concatenate([row, np.arange(N, dtype=np.int64)])
    deg = np.bincount(rows_all, minlength=N).astype(np.float64)
    dinv = deg ** -0.5
    norm = (dinv[row] * dinv[col]).astype(np.float32)
    dinv2 = (dinv * dinv).astype(np.float32)

    NBLK_TOT = CORES * NBLK
    blk_of_edge = row // BLK
    is_hi = (col >= SPL).astype(np.int64)
    seg_key = blk_of_edge * 2 + is_hi
    order = np.argsort(seg_key, kind="stable")
    rs, cs, ns_, sk = row[order], col[order], norm[order], seg_key[order]
    EE = rs.shape[0]

    seg_cnt = np.bincount(sk, minlength=NBLK_TOT * 2)
    seg_tiles = (seg_cnt + 127) // 128  # exact tiles per (block, half)
    t_lo_all = seg_tiles[0::2].reshape(CORES, NBLK)
    t_hi_all = seg_tiles[1::2].reshape(CORES, NBLK)

    seg_start = np.zeros(NBLK_TOT * 2 + 1, np.int64)
    np.cumsum(seg_cnt, out=seg_start[1:])
    pos_in_seg = np.arange(EE) - seg_start[sk]

    # Build per-core streams ordered chunk-major (chunks = consecutive blocks):
    #   idx streams:  for block: lo slots   (lo stream); same for hi
    #   onehot slabs: for block: lo tiles, hi tiles, self slab
    # Segments padded to tiles*128 slots; pads use idx 0 / onehot value 0.
    # SPMD: all cores share one program, so the tile structure (t_lo/t_hi)
    # is the per-block MAX across cores.
    t_lo_max = t_lo_all.max(axis=0)
    t_hi_max = t_hi_all.max(axis=0)
    XLB_PAD = NPC + (128 - BLK) + 8
    in_maps = []
    for k in range(CORES):
        t_lo = t_lo_max
        t_hi = t_hi_max
        TOT_TILES = int(t_lo.sum() + t_hi.sum()) + NBLK
        LO_SLOTS = int(t_lo.sum()) * 128
        HI_SLOTS = int(t_hi.sum()) * 128
        lo_stream = np.zeros(LO_SLOTS, np.int16)
        hi_stream = np.zeros(HI_SLOTS, np.int16)
        oh_arr = np.zeros((128, TOT_TILES, BLK), np.float32)

        slab_lo = np.zeros(NBLK, np.int64)
        slab_hi = np.zeros(NBLK, np.int64)
        slab_self = np.zeros(NBLK, np.int64)
        acc = 0
        for lb in range(NBLK):
            slab_lo[lb] = acc
            acc += t_lo[lb]
            slab_hi[lb] = acc
            acc += t_hi[lb]
            slab_self[lb] = acc
            acc += 1
        lo_base = np.zeros(NBLK, np.int64)
        hi_base = np.zeros(NBLK, np.int64)
        np.cumsum(t_lo[:-1] * 128, out=lo_base[1:])
        np.cumsum(t_hi[:-1] * 128, out=hi_base[1:])

        for half, (stream, base, slab) in enumerate(
            ((lo_stream, lo_base, slab_lo), (hi_stream, hi_base, slab_hi))
        ):
            m = (sk >= k * NBLK * 2) & (sk < (k + 1) * NBLK * 2) & (sk % 2 == half)
            idxs = np.nonzero(m)[0]
            if idxs.size == 0:
                continue
            lb = (sk[idxs] // 2) % NBLK
            pos = pos_in_seg[idxs]
            stream[base[lb] + pos] = (cs[idxs] - half * SPL).astype(np.int16)
            tile_in = pos // 128
            p = pos % 128
            r = (rs[idxs] - (sk[idxs] // 2) * BLK).astype(np.int64)
            oh_arr[p, slab[lb] + tile_in, r] = ns_[idxs]

        # self slabs: diagonal dinv^2 for this core's rows
        pp = np.arange(BLK)
        for lb in range(NBLK):
            gi = k * NPC + lb * BLK + pp
            oh_arr[pp, slab_self[lb], pp] = dinv2[gi]

        lo_idx = _pack_idx(lo_stream)
        hi_idx = _pack_idx(hi_stream)
        oh_bf = oh_arr.reshape(128, TOT_TILES * BLK).astype(ml_dtypes.bfloat16)

        xlb = np.zeros((XLB_PAD, D), ml_dtypes.bfloat16)
        xlb[:NPC] = x[k * NPC:(k + 1) * NPC].astype(ml_dtypes.bfloat16)

        in_maps.append({
            "x_bf": None,  # filled below
            "x_loc_bf": xlb,
            "lo_idx": lo_idx,
            "hi_idx": hi_idx,
            "oh_arr": oh_bf,
            "x_loc": x[k * NPC:(k + 1) * NPC].copy(),
            "w_mat": W.astype(ml_dtypes.bfloat16),
            "gamma": gamma,
            "beta": beta,
        })

    x_bf = np.zeros((XF_PAD, D), ml_dtypes.bfloat16)
    x_bf[:N] = x.astype(ml_dtypes.bfloat16)
    for k in range(CORES):
        in_maps[k]["x_bf"] = x_bf

    TOT_TILES = int(t_lo_max.sum() + t_hi_max.sum()) + NBLK
    params = (N, D, NPC, BLK, NBLK, SUP, SPL, XF_PAD, XLB_PAD,
              tuple(int(v) for v in t_lo_max), tuple(int(v) for v in t_hi_max),
              int(t_lo_max.sum()) * 8, int(t_hi_max.sum()) * 8, TOT_TILES)
    return params, in_maps


def get_nc(params):
    if params not in _CACHE:
        _CACHE[params] = _build_nc(*params)
    return _CACHE[params]


def run(params, in_maps, trace=False, **kw):
    from concourse.bass_utils import run_bass_kernel_spmd

    nc = get_nc(params)
    res = run_bass_kernel_spmd(nc, in_maps, list(range(CORES)), trace=trace, **kw)
    y = np.concatenate([res.results[k]["y_out"] for k in range(CORES)], axis=0)
    return y.astype(np.float32), res


def kernel(x, edge_index, W, b, gamma, beta):
    params, in_maps = prepare(x, edge_index, W, b, gamma, beta)
    y, _ = run(params, in_maps)
    return y


# revision 28
# speedup vs baseline: 1.0311x; 1.0311x over previous
"""GCNConv(flow=target_to_source) + BatchNorm + ReLU + residual, on 8 trn2 NeuronCores.

Math: with self-loops appended to the edge list,
    deg[i]   = #{e : row[e] == i}
    dinv     = deg ** -0.5
    norm_e   = dinv[row_e] * dinv[col_e]
    S[r]     = sum_{e: row[e]=r} norm_e * x[col_e]     (dma_gather + onehot-matmul)
    out      = S @ W                                   (W commutes past the aggregation)
    y        = relu((out - mean) * rsqrt(var + eps) * gamma + beta) + x
(b cancels inside BatchNorm, so it is dropped.)

v2 vs v1: no bf16 v-table is built on device.  The gather reads x_bf directly
and the full GCN norm (dinv[row]*dinv[col]) is folded into host-precomputed
"onehot" scatter matrices (bf16 values at (edge_slot, target_row), zero on pad
slots) — standard GCN edge-weight preprocessing.  This removes the serial
v-build phase and the DVE-hostile stride-0 is_equal onehot build (DVE runs at
1 elem/cycle when any operand has a stride-0 last dim).  Tile counts are exact
per (block, lo/hi) segment maxed across cores (SPMD shares one program).
Appended self-loops are not gathered: each block gets one "self" tile filled
by a plain strided DMA of the core's own rows plus a diagonal dinv^2 onehot
slab (-10% gather descriptors).  Each gather call is split across two SWDGE
queues (_QSPLIT=2) to overlap Q7 descriptor generation, the dominant gather
cost (~3ns/descriptor).

Sharding: nodes (rows) split across 8 cores; edges partitioned by destination
row so the scatter-add is core-local PSUM accumulation.  BN statistics go
through a [128,2] AllReduce.

dma_gather takes int16 indices, so x_bf is addressed as two halves
(lo: rows < SPLIT, hi: rows >= SPLIT); each block's edges are ordered
lo-cols-first.  Index buffers are packed in the HW layout: idx i at
(partition i%16, column i//16), replicated across the eight 16-partition
groups.
"""

import os
import sys

sys.path.insert(0, "/opt/trn_rl_repo")
os.environ.setdefault("MYCRO_LOCAL_CACHE", "1")

from contextlib import ExitStack

import ml_dtypes
import numpy as np

CORES = 8
BN_EPS = 1e-5
SPLIT = 32768
_REP = 1  # debug/timing: repeat the whole compute body inside one program
_FAKE_GATHER = False  # timing-only: plain DMA instead of dma_gather (wrong data)
_SKIP_CC = False      # timing-only: skip the AllReduce (wrong stats)
_QSPLIT = 2  # split each gather call into this many queue-parallel pieces
_NSLAB = 1   # final apply/store pipelining granularity (1 = monolithic)
_GBUFS = 2   # gather-pool depth (3 showed no robust win over 2)
_SINGLE_PACKET = False  # coalesce each gather's descriptor stream into one packet
_PSBUFS = 2  # ps_main depth (PSUM is 8 banks: 2x2 main + 2 stat + 2 misc)
_CACHE: dict = {}


def _pick_blk(npc: int) -> int:
    for blk in range(125, 0, -1):
        if npc % blk == 0:
            return blk
    raise ValueError(npc)


def _strided(ap_src, offset_elems, dims):
    import concourse.bass as bass

    return bass.AP(ap_src.tensor, offset_elems, [list(d) for d in dims])


def _build_nc(N, D, NPC, BLK, NBLK, SUP, SPL, XF_PAD, XLB_PAD,
              t_lo, t_hi, LO_COLS, HI_COLS, TOT_TILES):
    """t_lo/t_hi: per-local-block tile counts (tuples of len NBLK).

    Appended self-loops are NOT gathered: each block has one extra "self"
    tile filled by a plain strided DMA from x_loc_bf (the core's own rows),
    with a diagonal onehot slab carrying dinv[i]^2.  Slab order per block:
    lo tiles, hi tiles, self tile."""
    from concourse import bacc, bass, mybir, tile
    from concourse.masks import make_identity

    f32 = mybir.dt.float32
    bf16 = mybir.dt.bfloat16
    i16 = mybir.dt.int16

    nc = bacc.Bacc(
        "TRN2",
        target_bir_lowering=False,
        debug=False,
        enable_asserts=False,
        num_devices=CORES,
        num_swdge_queues=4,
    )

    x_bf = nc.dram_tensor("x_bf", [XF_PAD, D], bf16, kind="ExternalInput").ap()
    xlb_t = nc.dram_tensor("x_loc_bf", [XLB_PAD, D], bf16, kind="ExternalInput").ap()
    lo_t = nc.dram_tensor("lo_idx", [128, LO_COLS], i16, kind="ExternalInput").ap()
    hi_t = nc.dram_tensor("hi_idx", [128, HI_COLS], i16, kind="ExternalInput").ap()
    oh_t = nc.dram_tensor("oh_arr", [128, TOT_TILES * BLK], bf16, kind="ExternalInput").ap()
    xloc_t = nc.dram_tensor("x_loc", [NPC, D], f32, kind="ExternalInput").ap()
    w_t = nc.dram_tensor("w_mat", [D, D], bf16, kind="ExternalInput").ap()
    gamma_t = nc.dram_tensor("gamma", [D], f32, kind="ExternalInput").ap()
    beta_t = nc.dram_tensor("beta", [D], f32, kind="ExternalInput").ap()
    y_t = nc.dram_tensor("y_out", [NPC, D], f32, kind="ExternalOutput").ap()

    # per-chunk prefix offsets (slots are 128-aligned so /16 cols are exact)
    chunks = []
    lo_slot0 = hi_slot0 = oh_tile0 = 0
    for c0 in range(0, NBLK, SUP):
        blocks = list(range(c0, min(c0 + SUP, NBLK)))
        LT = sum(t_lo[b] for b in blocks)
        HT = sum(t_hi[b] for b in blocks)
        chunks.append((blocks, LT, HT, lo_slot0, hi_slot0, oh_tile0))
        lo_slot0 += LT * 128
        hi_slot0 += HT * 128
        oh_tile0 += LT + HT + len(blocks)  # + self slabs

    with tile.TileContext(nc) as tc, ExitStack() as ctx:
        const = ctx.enter_context(tc.tile_pool(name="const", bufs=1))
        gath = ctx.enter_context(tc.tile_pool(name="gath", bufs=_GBUFS))
        ohp = ctx.enter_context(tc.tile_pool(name="ohp", bufs=2))
        evp = ctx.enter_context(tc.tile_pool(name="evp", bufs=3))
        big = ctx.enter_context(tc.tile_pool(name="big", bufs=1))
        ps_main = ctx.enter_context(tc.tile_pool(name="ps_main", bufs=_PSBUFS, space="PSUM"))
        ps_stat = ctx.enter_context(tc.tile_pool(name="ps_stat", bufs=1, space="PSUM"))
        ps_misc = ctx.enter_context(tc.tile_pool(name="ps_misc", bufs=2, space="PSUM"))
        dram = ctx.enter_context(tc.tile_pool(name="dram", bufs=1, space="DRAM"))

        # ---- constants -----------------------------------------------------
        w_sb = const.tile([D, D], bf16)
        nc.sync.dma_start(w_sb[:], w_t[:])
        lo_sb = const.tile([128, LO_COLS], i16)
        nc.sync.dma_start(lo_sb[:], lo_t[:])
        hi_sb = const.tile([128, HI_COLS], i16)
        nc.sync.dma_start(hi_sb[:], hi_t[:])
        ones_sb = const.tile([128, 1], f32)
        nc.vector.memset(ones_sb[:], 1.0)
        onesrow_sb = const.tile([1, 128], f32)
        nc.vector.memset(onesrow_sb[:], 1.0)
        gb_sb = const.tile([128, 2], f32)
        nc.sync.dma_start(gb_sb[:, 0:1], gamma_t[:, None])
        nc.sync.dma_start(gb_sb[:, 1:2], beta_t[:, None])
        ident_sb = const.tile([128, 128], f32)
        make_identity(nc, ident_sb[:])

        # residual x, loaded early (independent of everything else)
        xl = big.tile([128, NBLK * D], f32)
        nc.sync.dma_start(
            xl[:BLK, :], _strided(xloc_t, 0, [[D, BLK], [BLK * D, NBLK], [1, D]])
        )

        for _rep in range(_REP):
            _sfx = f"_r{_rep}"
            out_all = big.tile([128, NBLK * D], f32, name="out_all" + _sfx)
            s1 = ps_stat.tile([128, 1], f32, tag="s1")
            s2 = ps_stat.tile([128, 1], f32, tag="s2")
            for ci, (blocks, LT, HT, lo_s0, hi_s0, oh_t0) in enumerate(chunks):
                nb = len(blocks)
                TT = LT + HT
                qq = ci * 2
                g = gath.tile([128, TT + nb, D], bf16, tag="g")
                # self tiles: block rows via plain strided DMA (all 128
                # partitions so no stale SBUF reaches the matmul; onehot is
                # zero beyond row BLK)
                nc.sync.dma_start(
                    g[:, TT:TT + nb, :],
                    _strided(xlb_t, blocks[0] * BLK * D,
                             [[D, 128], [BLK * D, nb], [1, D]]),
                )
                if _FAKE_GATHER:
                    nc.sync.dma_start(
                        g[:, 0:TT, :],
                        _strided(x_bf, 0, [[D, 128], [128 * D, TT], [1, D]]),
                    )
                else:
                    def issue(tile0, ntiles, src_ap, idx_sb, slot0, qbase, qstep):
                        pieces = _QSPLIT if ntiles >= _QSPLIT else max(ntiles, 1)
                        tb = 0
                        for pi in range(pieces):
                            nt = ntiles // pieces + (1 if pi < ntiles % pieces else 0)
                            if nt == 0:
                                continue
                            s0 = slot0 + tb * 128
                            nc.gpsimd.dma_gather(
                                g[:, tile0 + tb:tile0 + tb + nt, :],
                                src_ap,
                                idx_sb[:, s0 // 16:(s0 + nt * 128) // 16],
                                nt * 128,
                                nt * 128,
                                D,
                                single_packet=_SINGLE_PACKET,
                                queue_num=(qbase + pi * qstep) % 4,
                            )
                            tb += nt

                    if LT:
                        issue(0, LT, x_bf[0:SPL, :], lo_sb, lo_s0, qq, 2)
                    if HT:
                        issue(LT, HT, x_bf[SPL:XF_PAD, :], hi_sb, hi_s0, qq + 1, 2)
                oh = ohp.tile([128, (TT + nb) * BLK], bf16, tag="oh")
                nc.sync.dma_start(
                    oh[:], oh_t[:, oh_t0 * BLK:(oh_t0 + TT + nb) * BLK]
                )
                # onehot slabs are ordered block-major: for each block its lo
                # tiles then its hi tiles.
                slab = 0
                lo_base = 0
                hi_base = LT
                for bj, b in enumerate(blocks):
                    tl, th = t_lo[b], t_hi[b]
                    gidx = [lo_base + t for t in range(tl)] + \
                           [hi_base + t for t in range(th)] + [TT + bj]
                    lo_base += tl
                    hi_base += th
                    st = ps_main.tile([128, BLK], f32, tag="st")
                    for k, gi in enumerate(gidx):
                        nc.tensor.matmul(
                            out=st[:], lhsT=g[:, gi, :],
                            rhs=oh[:, slab * BLK:(slab + 1) * BLK],
                            start=(k == 0), stop=(k == len(gidx) - 1),
                        )
                        slab += 1
                    stb = evp.tile([128, BLK], bf16, tag="stb")
                    nc.vector.tensor_copy(out=stb[:], in_=st[:])
                    ow = ps_main.tile([BLK, D], f32, tag="ow")
                    nc.tensor.matmul(out=ow[:], lhsT=stb[:], rhs=w_sb[:],
                                     start=True, stop=True)
                    oslice = out_all[:BLK, b * D:(b + 1) * D]
                    nc.scalar.copy(out=oslice, in_=ow[:])
                    sq_s = evp.tile([128, D], f32, tag="sq")
                    nc.scalar.square(out=sq_s[:BLK, :], in_=oslice)
                    nc.tensor.matmul(
                        out=s1[:], lhsT=oslice, rhs=ones_sb[:BLK, :],
                        start=(b == 0), stop=(b == NBLK - 1),
                    )
                    nc.tensor.matmul(
                        out=s2[:], lhsT=sq_s[:BLK, :], rhs=ones_sb[:BLK, :],
                        start=(b == 0), stop=(b == NBLK - 1),
                    )

            # ---- BN stats AllReduce + affine params ----------------------------
            stat_sb = const.tile([128, 2], f32, name="stat_sb" + _sfx)
            nc.vector.tensor_copy(out=stat_sb[:, 0:1], in_=s1[:])
            nc.vector.tensor_copy(out=stat_sb[:, 1:2], in_=s2[:])
            cc_in = dram.tile([128, 2], f32)
            cc_out = dram.tile([128, 2], f32, addr_space="Shared")
            statg = const.tile([128, 2], f32, name="statg" + _sfx)
            if _SKIP_CC:
                nc.vector.tensor_copy(out=statg[:], in_=stat_sb[:])
            else:
                nc.sync.dma_start(cc_in[:], stat_sb[:])
                nc.gpsimd.collective_compute(
                    "AllReduce",
                    mybir.AluOpType.add,
                    replica_groups=[list(range(CORES))],
                    ins=[cc_in.opt()],
                    outs=[cc_out.opt()],
                )
                nc.sync.dma_start(statg[:], cc_out[:])

            invn = 1.0 / float(N)
            mean = const.tile([128, 1], f32, name="mean" + _sfx)
            nc.vector.tensor_scalar(
                out=mean[:], in0=statg[:, 0:1], scalar1=invn, scalar2=None,
                op0=mybir.AluOpType.mult,
            )
            vareps = const.tile([128, 1], f32, name="vareps" + _sfx)
            m2 = const.tile([128, 1], f32, name="m2" + _sfx)
            nc.vector.tensor_tensor(out=m2[:], in0=mean[:], in1=mean[:], op=mybir.AluOpType.mult)
            nc.vector.tensor_scalar(
                out=vareps[:], in0=statg[:, 1:2], scalar1=invn, scalar2=BN_EPS,
                op0=mybir.AluOpType.mult, op1=mybir.AluOpType.add,
            )
            nc.vector.tensor_tensor(
                out=vareps[:], in0=vareps[:], in1=m2[:], op=mybir.AluOpType.subtract
            )
            rec1 = const.tile([128, 1], f32, name="rec1" + _sfx)
            nc.vector.reciprocal(out=rec1[:], in_=vareps[:])
            rsq = const.tile([128, 1], f32, name="rsq" + _sfx)
            nc.scalar.sqrt(out=rsq[:], in_=rec1[:])
            ab_sb = const.tile([128, 2], f32, name="ab_sb" + _sfx)
            nc.vector.tensor_tensor(
                out=ab_sb[:, 0:1], in0=rsq[:], in1=gb_sb[:, 0:1], op=mybir.AluOpType.mult
            )
            tmb = const.tile([128, 1], f32, name="tmb" + _sfx)
            nc.vector.tensor_tensor(
                out=tmb[:], in0=mean[:], in1=ab_sb[:, 0:1], op=mybir.AluOpType.mult
            )
            nc.vector.tensor_tensor(
                out=ab_sb[:, 1:2], in0=gb_sb[:, 1:2], in1=tmb[:], op=mybir.AluOpType.subtract
            )

            def bcast_col(col_ap, nm):
                tp = ps_misc.tile([128, 128], f32, tag="m")
                nc.tensor.transpose(out=tp[:1, :], in_=col_ap, identity=ident_sb[:])
                rowt = const.tile([1, 128], f32, name=f"rowt_{nm}" + _sfx)
                nc.vector.tensor_copy(out=rowt[:], in_=tp[:1, :])
                bc_ps = ps_misc.tile([128, 128], f32, tag="m")
                nc.tensor.matmul(out=bc_ps[:], lhsT=onesrow_sb[:], rhs=rowt[:], start=True, stop=True)
                bc = const.tile([128, 128], f32, name=f"bc_{nm}" + _sfx)
                nc.vector.tensor_copy(out=bc[:], in_=bc_ps[:])
                return bc

            a_bc = bcast_col(ab_sb[:, 0:1], "a")
            b_bc = bcast_col(ab_sb[:, 1:2], "b")

            # ---- final apply: y = relu(out*A + B) + x --------------------------
            # per-slab so each slab's y store overlaps the next slab's apply
            SB = NBLK // _NSLAB if NBLK % _NSLAB == 0 else NBLK
            for s0 in range(0, NBLK, SB):
                cols = slice(s0 * D, (s0 + SB) * D)
                a_rep = _strided(a_bc[:], 0, [[a_bc[:].ap[0][0], BLK], [0, SB], [1, D]])
                b_rep = _strided(b_bc[:], 0, [[b_bc[:].ap[0][0], BLK], [0, SB], [1, D]])
                oc = out_all[:BLK, cols]
                nc.vector.tensor_tensor(out=oc, in0=oc, in1=a_rep, op=mybir.AluOpType.mult)
                nc.vector.tensor_tensor(out=oc, in0=oc, in1=b_rep, op=mybir.AluOpType.add)
                nc.vector.tensor_scalar(
                    out=oc, in0=oc, scalar1=0.0, scalar2=None, op0=mybir.AluOpType.max,
                )
                nc.vector.tensor_tensor(
                    out=oc, in0=oc, in1=xl[:BLK, cols], op=mybir.AluOpType.add,
                )
                nc.sync.dma_start(
                    _strided(y_t, s0 * BLK * D, [[D, BLK], [BLK * D, SB], [1, D]]), oc
                )

    nc.compile()
    return nc


def _pack_idx(stream):
    """Pack a flat slot stream (len multiple of 128) into the dma_gather int16
    layout: idx i -> (partition i%16, col i//16), replicated across the 8
    groups of 16 partitions.  Returns [128, len//16] int16."""
    assert len(stream) % 128 == 0
    grid = stream.reshape(-1, 16).T  # [16, ncols]
    return np.tile(grid, (8, 1))


def prepare(x, edge_index, W, b, gamma, beta):
    x = np.asarray(x, np.float32)
    W = np.asarray(W, np.float32)
    gamma = np.asarray(gamma, np.float32)
    beta = np.asarray(beta, np.float32)
    N, D = x.shape
    assert N % CORES == 0
    NPC = N // CORES
    BLK = _pick_blk(NPC)
    NBLK = NPC // BLK
    SUP = 1
    for s in (5, 4, 3, 2):
        if NBLK % s == 0:
            SUP = s
            break
    SPL = min(SPLIT, N)
    XF_PAD = ((N + 127) // 128) * 128

    row = np.asarray(edge_index[0]).astype(np.int64)
    col = np.asarray(edge_index[1]).astype(np.int64)
    # degree counts the appended self-loops, but the loops themselves are
    # handled by a per-block DMA + diagonal onehot slab (not gathered)
    rows_all = np.concatenate([row, np.arange(N, dtype=np.int64)])
    deg = np.bincount(rows_all, minlength=N).astype(np.float64)
    dinv = deg ** -0.5
    norm = (dinv[row] * dinv[col]).astype(np.float32)
    dinv2 = (dinv * dinv).astype(np.float32)

    NBLK_TOT = CORES * NBLK
    blk_of_edge = row // BLK
    is_hi = (col >= SPL).astype(np.int64)
    seg_key = blk_of_edge * 2 + is_hi
    order = np.argsort(seg_key, kind="stable")
    rs, cs, ns_, sk = row[order], col[order], norm[order], seg_key[order]
    EE = rs.shape[0]

    seg_cnt = np.bincount(sk, minlength=NBLK_TOT * 2)
    seg_tiles = (seg_cnt + 127) // 128  # exact tiles per (block, half)
    t_lo_all = seg_tiles[0::2].reshape(CORES, NBLK)
    t_hi_all = seg_tiles[1::2].reshape(CORES, NBLK)

    seg_start = np.zeros(NBLK_TOT * 2 + 1, np.int64)
    np.cumsum(seg_cnt, out=seg_start[1:])
    pos_in_seg = np.arange(EE) - seg_start[sk]

    # Build per-core streams ordered chunk-major (chunks = consecutive blocks):
    #   idx streams:  for block: lo slots   (lo stream); same for hi
    #   onehot slabs: for block: lo tiles, hi tiles, self slab
    # Segments padded to tiles*128 slots; pads use idx 0 / onehot value 0.
    # SPMD: all cores share one program, so the tile structure (t_lo/t_hi)
    # is the per-block MAX across cores.
    t_lo_max = t_lo_all.max(axis=0)
    t_hi_max = t_hi_all.max(axis=0)
    XLB_PAD = NPC + (128 - BLK) + 8
    in_maps = []
    for k in range(CORES):
        t_lo = t_lo_max
        t_hi = t_hi_max
        TOT_TILES = int(t_lo.sum() + t_hi.sum()) + NBLK
        LO_SLOTS = int(t_lo.sum()) * 128
        HI_SLOTS = int(t_hi.sum()) * 128
        lo_stream = np.zeros(LO_SLOTS, np.int16)
        hi_stream = np.zeros(HI_SLOTS, np.int16)
        oh_arr = np.zeros((128, TOT_TILES, BLK), np.float32)

        slab_lo = np.zeros(NBLK, np.int64)
        slab_hi = np.zeros(NBLK, np.int64)
        slab_self = np.zeros(NBLK, np.int64)
        acc = 0
        for lb in range(NBLK):
            slab_lo[lb] = acc
            acc += t_lo[lb]
            slab_hi[lb] = acc
            acc += t_hi[lb]
            slab_self[lb] = acc
            acc += 1
        lo_base = np.zeros(NBLK, np.int64)
        hi_base = np.zeros(NBLK, np.int64)
        np.cumsum(t_lo[:-1] * 128, out=lo_base[1:])
        np.cumsum(t_hi[:-1] * 128, out=hi_base[1:])

        for half, (stream, base, slab) in enumerate(
            ((lo_stream, lo_base, slab_lo), (hi_stream, hi_base, slab_hi))
        ):
            m = (sk >= k * NBLK * 2) & (sk < (k + 1) * NBLK * 2) & (sk % 2 == half)
            idxs = np.nonzero(m)[0]
            if idxs.size == 0:
                continue
            lb = (sk[idxs] // 2) % NBLK
            pos = pos_in_seg[idxs]
            stream[base[lb] + pos] = (cs[idxs] - half * SPL).astype(np.int16)
            tile_in = pos // 128
            p = pos % 128
            r = (rs[idxs] - (sk[idxs] // 2) * BLK).astype(np.int64)
            oh_arr[p, slab[lb] + tile_in, r] = ns_[idxs]

        # self slabs: diagonal dinv^2 for this core's rows
        pp = np.arange(BLK)
        for lb in range(NBLK):
            gi = k * NPC + lb * BLK + pp
            oh_arr[pp, slab_self[lb], pp] = dinv2[gi]

        lo_idx = _pack_idx(lo_stream)
        hi_idx = _pack_idx(hi_stream)
        oh_bf = oh_arr.reshape(128, TOT_TILES * BLK).astype(ml_dtypes.bfloat16)

        xlb = np.zeros((XLB_PAD, D), ml_dtypes.bfloat16)
        xlb[:NPC] = x[k * NPC:(k + 1) * NPC].astype(ml_dtypes.bfloat16)

        in_maps.append({
            "x_bf": None,  # filled below
            "x_loc_bf": xlb,
            "lo_idx": lo_idx,
            "hi_idx": hi_idx,
            "oh_arr": oh_bf,
            "x_loc": x[k * NPC:(k + 1) * NPC].copy(),
            "w_mat": W.astype(ml_dtypes.bfloat16),
            "gamma": gamma,
            "beta": beta,
        })

    x_bf = np.zeros((XF_PAD, D), ml_dtypes.bfloat16)
    x_bf[:N] = x.astype(ml_dtypes.bfloat16)
    for k in range(CORES):
        in_maps[k]["x_bf"] = x_bf

    TOT_TILES = int(t_lo_max.sum() + t_hi_max.sum()) + NBLK
    params = (N, D, NPC, BLK, NBLK, SUP, SPL, XF_PAD, XLB_PAD,
              tuple(int(v) for v in t_lo_max), tuple(int(v) for v in t_hi_max),
              int(t_lo_max.sum()) * 8, int(t_hi_max.sum()) * 8, TOT_TILES)
    return params, in_maps


def get_nc(params):
    if params not in _CACHE:
        _CACHE[params] = _build_nc(*params)
    return _CACHE[params]


def run(params, in_maps, trace=False, **kw):
    from concourse.bass_utils import run_bass_kernel_spmd

    nc = get_nc(params)
    res = run_bass_kernel_spmd(nc, in_maps, list(range(CORES)), trace=trace, **kw)
    y = np.concatenate([res.results[k]["y_out"] for k in range(CORES)], axis=0)
    return y.astype(np.float32), res


def kernel(x, edge_index, W, b, gamma, beta):
    params, in_maps = prepare(x, edge_index, W, b, gamma, beta)
    y, _ = run(params, in_maps)
    return y
